# revision 1
# baseline (speedup 1.0000x reference)
"""CFNet interaction block on 8 trn2 NeuronCores (SPMD bass/tile kernel).

Strategy (per core c of 8, SPMD — one program, per-core input data):
  - Edges are sharded by ATOM ranges: core c owns atoms [c*NA, (c+1)*NA) and
    all edges whose (sorted) seg_i falls in that range.  Output atom slices
    are disjoint; the host concatenates them (no device collective).
  - Within a core, edges are reordered: [h0 edges (idx_j < 32768), then h1]
    (dma_gather indices are int16), each group seg-sorted, padded with dump
    edges to a fixed tile count.  Tile 0 is all-dump => LS rows 0..SMAX-1
    are zero (used as the level-2 zero row).
  - Edge pipeline per 512-edge block: SWDGE cast-load dijk fp32->bf16,
    PE-transpose to [k, e], mm1 (W1), ssp (Exp + Ln(0.5+0.5e)), mm2 (W2)
    -> w [e, f], ssp, wf = w * f (f gathered from xf = x @ Win), and a
    segment-sum matmul LS_t[slot, f] = S_t^T @ wf into an LS table in DRAM.
  - Level 2: conv[a] = sum of the (<= KRUN) LS rows of atom a, via KRUN
    dma_gather passes over the LS table + DVE adds.
  - Tail: conv -> transpose -> z3^T = Wout^T conv^T -> ssp -> h^T
    -> v = h @ Wd -> y = x + v, written as per-core atom slices.

ssp(x) = softplus(x) - log 2 = Ln(0.5 + 0.5*Exp(x)) exactly (2 ACT passes).
All matmuls run in bf16 (fp32 PSUM accumulate); measured end-to-end error
vs the fp32 reference ~5e-3 relative.
"""

import os
import sys
import math

import numpy as np
import ml_dtypes

sys.path.insert(0, "/opt/trn_rl_repo")

import concourse.bacc as bacc
import concourse.mybir as mybir
from concourse import tile
from concourse.bass_utils import run_bass_kernel_spmd

dt = mybir.dt
AF = mybir.ActivationFunctionType

N_CORES = 8
HALF = 32768          # int16 gather index limit
SMAX = 32             # max distinct segments per 128-edge tile
SPAD = 32             # S slots padded so mm3 writes full 32-partition groups
KH = 4                # conv gather passes per LS half-table
GCALL = 1024          # indices per dma_gather call
BF16 = ml_dtypes.bfloat16


def _ceil(a, b):
    return -(-a // b)


class Plan:
    """Shape-derived (value-independent) structure constants."""

    def __init__(self, n_atoms, n_edges, n_in):
        assert n_atoms % N_CORES == 0
        self.n_atoms, self.n_edges, self.n_in = n_atoms, n_edges, n_in
        self.NA = n_atoms // N_CORES
        self.NA_PAD = _ceil(self.NA, GCALL) * GCALL          # conv gather grid
        self.NCONV_CALL = self.NA_PAD // GCALL
        # worst-case per-core edges: mean + 6 sigma (Poisson-ish)
        mean_e = n_edges / N_CORES
        emax = mean_e + 6.0 * math.sqrt(mean_e) + 256
        frac0 = HALF / n_atoms if n_atoms > HALF else 1.0
        # tiles per half-bucket (each a multiple of 8 tiles = 1 gather call),
        # +1 dump tile at the start of h0; sized for emax with 4-sigma split
        # wobble.
        sig = math.sqrt(mean_e * frac0 * (1 - frac0)) if frac0 < 1.0 else 0.0
        e0 = emax * frac0 + 4 * sig
        e1 = emax * (1 - frac0) + 4 * sig
        # multiples of 16 tiles: one dijk cast-load = 16 tiles, one f-gather
        # call = 8 tiles
        self.NT0 = int(_ceil(int(e0 / 128 + 1), 16) * 16)
        self.NT1 = int(_ceil(int(e1 / 128 + 1), 16) * 16) if frac0 < 1.0 else 0
        self.T = self.NT0 + self.NT1
        assert self.T % 4 == 0
        self.E_PC = self.T * 128
        self.NBLK = self.T // 4                              # 512-edge blocks
        self.R = self.T * SPAD                               # LS rows (32/tile)
        self.RH = self.R // 2                                # rows per half table
        assert self.RH < 32767, "LS half-table row ids must fit int16"
        assert self.T % 2 == 0
        self.NFCALL = self.T // 8                            # f-gather calls
        self.KC = [n_in - 128 * i if n_in - 128 * i < 128 else 128
                   for i in range(_ceil(n_in, 128))]         # k-chunk sizes
        self.NKC = len(self.KC)
        # x blocks for the xf phase
        self.NXBLK = _ceil(n_atoms, 512)
        self.NX_PAD = self.NXBLK * 512


def shard_inputs(plan, x, dijk, idx_j, seg_i):
    """Host-side preprocessing. Returns per-core input dicts (np arrays)."""
    p = plan
    n_atoms = p.n_atoms
    idx_j = np.asarray(idx_j).astype(np.int64)
    seg_i = np.asarray(seg_i).astype(np.int64)
    bounds = np.searchsorted(seg_i, np.arange(N_CORES + 1) * p.NA)

    per_core = []
    for c in range(N_CORES):
        lo, hi = bounds[c], bounds[c + 1]
        ej = idx_j[lo:hi]
        es = seg_i[lo:hi] - c * p.NA          # local segment ids [0, NA)
        h1m = ej >= HALF
        order = np.concatenate([np.nonzero(~h1m)[0], np.nonzero(h1m)[0]])
        n0 = int((~h1m).sum())
        n1 = int(h1m.sum())
        assert n0 <= (p.NT0 - 1) * 128, (c, n0, p.NT0)
        assert n1 <= p.NT1 * 128, (c, n1, p.NT1)

        # edge permutation with -1 = dump; tile 0 all-dump
        perm = np.full(p.E_PC, -1, dtype=np.int64)
        perm[128:128 + n0] = order[:n0]
        perm[p.NT0 * 128: p.NT0 * 128 + n1] = order[n0:]
        valid = perm >= 0

        # dijk shard (dump rows zero)
        dsh = np.zeros((p.E_PC, p.n_in), dtype=np.float32)
        dsh[valid] = dijk[lo + perm[valid]]

        # per-edge local segment (-1 for dump)
        eseg = np.full(p.E_PC, -1, dtype=np.int64)
        eseg[valid] = es[perm[valid]]

        # per-edge gather index (16-bit domain), dump -> 0
        egidx = np.zeros(p.E_PC, dtype=np.int64)
        egidx[valid] = ej[perm[valid]]
        egidx[p.NT0 * 128:] -= HALF * (egidx[p.NT0 * 128:] >= 0).astype(np.int64)
        egidx[p.NT0 * 128:][~valid[p.NT0 * 128:]] = 0
        egidx[~valid] = 0
        assert egidx.min() >= 0 and egidx.max() < HALF

        # S pages + LS row -> atom map
        tseg = eseg.reshape(p.T, 128)
        S = np.zeros((p.T, 128, SPAD), dtype=np.float32)
        row_atom = np.full(p.R, -1, dtype=np.int64)   # LS row -> local atom
        for t in range(p.T):
            segs = tseg[t]
            vmask = segs >= 0
            if not vmask.any():
                continue
            uniq, inv = np.unique(segs[vmask], return_inverse=True)
            assert len(uniq) <= SMAX, (c, t, len(uniq))
            S[t, np.nonzero(vmask)[0], inv] = 1.0
            row_atom[t * SPAD: t * SPAD + len(uniq)] = uniq
        # pages of 8 tiles: [page, 128, 8*SMAX]
        S_pages = (
            S.reshape(p.T // 8, 8, 128, SPAD)
            .transpose(0, 2, 1, 3)
            .reshape(p.T // 8, 128, 8 * SPAD)
            .astype(BF16)
        )

        # per-atom LS row lists, split by table half (int16 limit)
        rows_of = [([], []) for _ in range(p.NA)]
        nz = np.nonzero(row_atom >= 0)[0]
        for r in nz:
            h = int(r >= p.RH)
            rows_of[row_atom[r]][h].append(r - h * p.RH)
        conv_idx = np.full((2 * KH, p.NA_PAD), 16, dtype=np.int64)  # 16 = zero row
        for a, (ra, rb) in enumerate(rows_of):
            assert len(ra) <= KH and len(rb) <= KH, (c, a, len(ra), len(rb))
            conv_idx[: len(ra), a] = ra
            conv_idx[KH: KH + len(rb), a] = rb

        per_core.append(
            dict(
                dijk_sh=dsh,
                f_idx=_wrap_idx(egidx.reshape(p.NFCALL, GCALL)),
                s_pages=S_pages,
                conv_idx=_wrap_idx(conv_idx.reshape(2 * KH * p.NCONV_CALL, GCALL)),
                xslice=_pad_rows(x[c * p.NA:(c + 1) * p.NA], p.NA_PAD),
            )
        )
    return per_core


def _wrap_idx(idx2d):
    """[ncalls, 1024] int -> [ncalls, 128, 64] int16 SWDGE layout."""
    ncalls = idx2d.shape[0]
    w = idx2d.astype(np.int16).reshape(ncalls, GCALL // 16, 16)
    w = np.transpose(w, (0, 2, 1))           # [ncalls, 16, 64]
    return np.ascontiguousarray(np.tile(w, (1, 8, 1)))  # [ncalls, 128, 64]


def _pad_rows(a, n):
    out = np.zeros((n,) + a.shape[1:], dtype=a.dtype)
    out[: a.shape[0]] = a
    return out


def build_program(p):
    nc = bacc.Bacc(None, target_bir_lowering=False)

    n_in = p.n_in
    # ---- dram parameters ----
    x_pad = nc.declare_dram_parameter("x_pad", [p.NX_PAD, 128], dt.float32, isOutput=False)
    xslice = nc.declare_dram_parameter("xslice", [p.NA_PAD, 128], dt.float32, isOutput=False)
    dijk_sh = nc.declare_dram_parameter("dijk_sh", [p.E_PC, n_in], dt.float32, isOutput=False)
    w1b = nc.declare_dram_parameter("w1b", [n_in, 128], dt.bfloat16, isOutput=False)
    w2b = nc.declare_dram_parameter("w2b", [128, 128], dt.bfloat16, isOutput=False)
    winb = nc.declare_dram_parameter("winb", [128, 128], dt.bfloat16, isOutput=False)
    woutb = nc.declare_dram_parameter("woutb", [128, 128], dt.bfloat16, isOutput=False)
    wdb = nc.declare_dram_parameter("wdb", [128, 128], dt.bfloat16, isOutput=False)
    ident_bf = nc.declare_dram_parameter("ident_bf", [128, 128], dt.bfloat16, isOutput=False)
    ident_f32 = nc.declare_dram_parameter("ident_f32", [128, 128], dt.float32, isOutput=False)
    s_pages = nc.declare_dram_parameter("s_pages", [p.T // 8, 128, 8 * SPAD], dt.bfloat16, isOutput=False)
    f_idx = nc.declare_dram_parameter("f_idx", [p.NFCALL, 128, GCALL // 16], dt.int16, isOutput=False)
    conv_idx = nc.declare_dram_parameter("conv_idx", [2 * KH * p.NCONV_CALL, 128, GCALL // 16], dt.int16, isOutput=False)

    y_out = nc.declare_dram_parameter("y_out", [p.NA_PAD, 128], dt.float32, isOutput=True)
    v_out = nc.declare_dram_parameter("v_out", [p.NA_PAD, 128], dt.float32, isOutput=True)

    # ---- internal dram ----
    xf_dram = nc.dram_tensor("xf_dram", [p.NX_PAD, 128], dt.float32)
    ls_dram = nc.dram_tensor("ls_dram", [p.R, 128], dt.float32)

    with tile.TileContext(nc) as tc:
        with (
            tc.tile_pool(name="const", bufs=1) as constp,
            tc.tile_pool(name="xph", bufs=3) as xph,
            tc.tile_pool(name="dload", bufs=2) as dload,
            tc.tile_pool(name="work", bufs=3) as work,
            tc.tile_pool(name="fbuf", bufs=3) as fbufp,
            tc.tile_pool(name="conv", bufs=1) as convp,
            tc.tile_pool(name="tail", bufs=3) as tailp,
            tc.tile_pool(name="psum", bufs=2, space="PSUM") as psum,
        ):
            # ---- constants ----
            identb = constp.tile([128, 128], dt.bfloat16)
            nc.sync.dma_start(out=identb[:], in_=ident_bf[:, :])
            identf = constp.tile([128, 128], dt.float32)
            nc.sync.dma_start(out=identf[:], in_=ident_f32[:, :])
            half_c = constp.tile([128, 1], dt.float32)
            nc.gpsimd.memset(half_c[:], 0.5)
            w1sb = []
            for kc in range(p.NKC):
                kn = p.KC[kc]
                t = constp.tile([128, 128], dt.bfloat16, name=f"w1sb{kc}")
                nc.sync.dma_start(out=t[:kn, :], in_=w1b[kc * 128: kc * 128 + kn, :])
                w1sb.append(t)
            w2sb = constp.tile([128, 128], dt.bfloat16)
            nc.sync.dma_start(out=w2sb[:], in_=w2b[:, :])
            winsb = constp.tile([128, 128], dt.bfloat16)
            nc.sync.dma_start(out=winsb[:], in_=winb[:, :])
            woutsb = constp.tile([128, 128], dt.bfloat16)
            nc.sync.dma_start(out=woutsb[:], in_=woutb[:, :])
            wdsb = constp.tile([128, 128], dt.bfloat16)
            nc.sync.dma_start(out=wdsb[:], in_=wdb[:, :])

            # ---- phase 0: xf = x @ Win -> xf_dram ----
            for g in range(p.NXBLK):
                xb = xph.tile([128, 4, 128], dt.bfloat16, tag="xb")
                nc.gpsimd.dma_start(
                    out=xb[:],
                    in_=x_pad[g * 512:(g + 1) * 512, :].rearrange(
                        "(i pp) b -> pp i b", pp=128
                    ),
                )
                xT_ps = psum.tile([128, 512], dt.bfloat16, tag="pst")
                for i in range(4):
                    nc.tensor.transpose(
                        xT_ps[:, i * 128:(i + 1) * 128], xb[:, i, :], identb[:]
                    )
                xT = xph.tile([128, 512], dt.bfloat16, tag="xT")
                nc.vector.tensor_copy(xT[:], xT_ps[:])
                xf_ps = psum.tile([128, 512], dt.float32, tag="psm")
                for i in range(4):
                    nc.tensor.matmul(
                        xf_ps[:, i * 128:(i + 1) * 128],
                        xT[:, i * 128:(i + 1) * 128],
                        winsb[:],
                        start=True,
                        stop=True,
                    )
                # NOTE: xf_ps slice i holds xf rows [g*512+i*128, +128) as
                # [128 a, 128 f]... but written as [128 a(part), 128 f] per
                # slice side by side -> [128, 4, 128] a-interleaved
                xf_sb = xph.tile([128, 4, 128], dt.float32, tag="xf_sb")
                nc.scalar.copy(xf_sb[:, 0, :], xf_ps[:, 0:128])
                nc.scalar.copy(xf_sb[:, 1, :], xf_ps[:, 128:256])
                nc.vector.tensor_copy(xf_sb[:, 2, :], xf_ps[:, 256:384])
                nc.vector.tensor_copy(xf_sb[:, 3, :], xf_ps[:, 384:512])
                nc.sync.dma_start(
                    out=xf_dram[g * 512:(g + 1) * 512, :].rearrange(
                        "(i pp) f -> pp i f", pp=128
                    ),
                    in_=xf_sb[:],
                )

            # wait: matmul slice i computes lhsT.T @ rhs with lhsT =
            # xT[:, i*128:+128] = (x block rows i*128..)^T  [b, a], rhs = Win
            # -> out [a, f] for atoms g*512 + i*128 + [0,128)  -- correct.

            # ---- phase A: edges ----
            LOAD_BLKS = 4  # 4 blocks of 512 edges per cast-load (2048 edges)
            n_loads = p.NBLK // LOAD_BLKS
            assert p.NBLK % LOAD_BLKS == 0

            for ld in range(n_loads):
                dblk = dload.tile([128, 16, n_in], dt.bfloat16, tag="dblk")
                e0 = ld * 2048
                nc.gpsimd.dma_start(
                    out=dblk[:],
                    in_=dijk_sh[e0: e0 + 2048, :].rearrange(
                        "(i pp) k -> pp i k", pp=128
                    ),
                )
                for sb in range(LOAD_BLKS):
                    b = ld * LOAD_BLKS + sb  # global 512-edge block id
                    # -- f gather: one call per 2 blocks (8 tiles) --
                    if b % 2 == 0:
                        call = b // 2
                        fb = fbufp.tile([128, 8, 128], dt.float32, tag="fb")
                        idxt = fbufp.tile([128, GCALL // 16], dt.int16, tag="fidx")
                        nc.sync.dma_start(out=idxt[:], in_=f_idx[call, :, :])
                        probe = fbufp.tile([16, 16], dt.int16, tag="fprobe")
                        nc.gpsimd.tensor_copy(probe[:], idxt[:16, :16])
                        src = xf_dram[:, :] if call < p.NT0 // 8 else xf_dram[HALF:, :]
                        nc.gpsimd.dma_gather(
                            fb[:], src, idxt[:], GCALL, GCALL, 128,
                            single_packet=False,
                        )
                        cur_fb = fb
                    # -- S page: one per 2 blocks --
                    if b % 2 == 0:
                        spg = fbufp.tile([128, 8 * SPAD], dt.bfloat16, tag="spg")
                        nc.sync.dma_start(out=spg[:], in_=s_pages[b // 2, :, :])
                        cur_spg = spg

                    # -- transposes: dijk block -> dT chunks [k, 512e] --
                    dT = []
                    for kc in range(p.NKC):
                        kn = p.KC[kc]
                        tps = psum.tile([128, 512], dt.bfloat16, tag="pst")
                        for i in range(4):
                            nc.tensor.transpose(
                                tps[:kn, i * 128:(i + 1) * 128],
                                dblk[:, sb * 4 + i, kc * 128: kc * 128 + kn],
                                identb[:],
                            )
                        tsb = work.tile([128, 512], dt.bfloat16, tag=f"dT{kc}")
                        if kc == 0:
                            nc.scalar.copy(tsb[:kn, :], tps[:kn, :])
                        else:
                            nc.vector.tensor_copy(tsb[:kn, :], tps[:kn, :])
                        dT.append(tsb)

                    # -- mm1: t1^T [f1, 512e] --
                    t1_ps = psum.tile([128, 512], dt.float32, tag="psm")
                    for kc in range(p.NKC):
                        kn = p.KC[kc]
                        nc.tensor.matmul(
                            t1_ps[:],
                            w1sb[kc][:kn, :],
                            dT[kc][:kn, :],
                            start=(kc == 0),
                            stop=(kc == p.NKC - 1),
                        )
                    # -- ssp1 --
                    e1 = work.tile([128, 512], dt.float32, tag="e1")
                    nc.scalar.activation(e1[:], t1_ps[:], AF.Exp)
                    t1s = work.tile([128, 512], dt.bfloat16, tag="t1s")
                    nc.scalar.activation(
                        t1s[:], e1[:], AF.Ln, bias=half_c[:], scale=half_c[:]
                    )
                    # -- mm2: w [512e, f2] in one psum bank --
                    w_ps = psum.tile([128, 512], dt.float32, tag="psm")
                    for i in range(4):
                        nc.tensor.matmul(
                            w_ps[:, i * 128:(i + 1) * 128],
                            t1s[:, i * 128:(i + 1) * 128],
                            w2sb[:],
                            start=True,
                            stop=True,
                        )
                    # -- sspw --
                    ew = work.tile([128, 512], dt.float32, tag="ew")
                    nc.scalar.activation(ew[:], w_ps[:], AF.Exp)
                    wt = work.tile([128, 512], dt.float32, tag="wt")
                    nc.scalar.activation(
                        wt[:], ew[:], AF.Ln, bias=half_c[:], scale=half_c[:]
                    )
                    # -- wf = w * f (gpsimd), out bf16 --
                    wf = work.tile([128, 512], dt.bfloat16, tag="wf")
                    fslice = cur_fb[:, (b % 2) * 4:(b % 2) * 4 + 4, :].rearrange(
                        "pp i f -> pp (i f)"
                    )
                    nc.gpsimd.tensor_tensor(
                        wf[:], wt[:], fslice, mybir.AluOpType.mult
                    )
                    # -- mm3: LS tiles [SMAX, 128f] x4 into one psum --
                    ls_ps = psum.tile([128, 128], dt.float32, tag="psl")
                    for i in range(4):
                        toff = (sb * 4 + i) % 8
                        nc.tensor.matmul(
                            ls_ps[32 * i: 32 * i + SPAD, :],
                            cur_spg[:, toff * SPAD:(toff + 1) * SPAD],
                            wf[:, i * 128:(i + 1) * 128],
                            start=True,
                            stop=True,
                            tile_position=(0, 32 * i),
                        )
                    ls_sb = work.tile([128, 128], dt.float32, tag="ls_sb")
                    nc.vector.tensor_copy(ls_sb[:], ls_ps[:])
                    nc.sync.dma_start(
                        out=ls_dram[b * 128:(b + 1) * 128, :], in_=ls_sb[:]
                    )

            # ---- phase B: conv gathers + tail ----
            conv = convp.tile([128, p.NA_PAD // 128, 128], dt.float32)
            for k in range(2 * KH):
                ls_src = ls_dram[: p.RH, :] if k < KH else ls_dram[p.RH:, :]
                for j in range(p.NCONV_CALL):
                    cid = k * p.NCONV_CALL + j
                    idxt = tailp.tile([128, GCALL // 16], dt.int16, tag="cidx")
                    nc.sync.dma_start(out=idxt[:], in_=conv_idx[cid, :, :])
                    probe = tailp.tile([16, 16], dt.int16, tag="cprobe")
                    nc.gpsimd.tensor_copy(probe[:], idxt[:16, :16])
                    if k == 0:
                        nc.gpsimd.dma_gather(
                            conv[:, j * 8:(j + 1) * 8, :], ls_src,
                            idxt[:], GCALL, GCALL, 128, single_packet=False,
                        )
                    else:
                        tmp = tailp.tile([128, 8, 128], dt.float32, tag="ctmp")
                        nc.gpsimd.dma_gather(
                            tmp[:], ls_src,
                            idxt[:], GCALL, GCALL, 128, single_packet=False,
                        )
                        nc.vector.tensor_tensor(
                            conv[:, j * 8:(j + 1) * 8, :],
                            conv[:, j * 8:(j + 1) * 8, :],
                            tmp[:],
                            mybir.AluOpType.add,
                        )

            for ch in range(p.NA_PAD // 128):
                cT_ps = psum.tile([128, 128], dt.float32, tag="pst")
                nc.tensor.transpose(cT_ps[:], conv[:, ch, :], identf[:])
                cT = tailp.tile([128, 128], dt.bfloat16, tag="cT")
                nc.vector.tensor_copy(cT[:], cT_ps[:])
                z3_ps = psum.tile([128, 128], dt.float32, tag="psm")
                nc.tensor.matmul(z3_ps[:], woutsb[:], cT[:], start=True, stop=True)
                e3 = tailp.tile([128, 128], dt.float32, tag="e3")
                nc.scalar.activation(e3[:], z3_ps[:], AF.Exp)
                hT = tailp.tile([128, 128], dt.bfloat16, tag="hT")
                nc.scalar.activation(
                    hT[:], e3[:], AF.Ln, bias=half_c[:], scale=half_c[:]
                )
                v_ps = psum.tile([128, 128], dt.float32, tag="psl")
                nc.tensor.matmul(v_ps[:], hT[:], wdsb[:], start=True, stop=True)
                v_sb = tailp.tile([128, 128], dt.float32, tag="v_sb")
                nc.vector.tensor_copy(v_sb[:], v_ps[:])
                nc.sync.dma_start(
                    out=v_out[ch * 128:(ch + 1) * 128, :], in_=v_sb[:]
                )
                xs = tailp.tile([128, 128], dt.float32, tag="xs")
                nc.sync.dma_start(
                    out=xs[:], in_=xslice[ch * 128:(ch + 1) * 128, :]
                )
                y_sb = tailp.tile([128, 128], dt.float32, tag="y_sb")
                nc.vector.tensor_tensor(
                    y_sb[:], v_sb[:], xs[:], mybir.AluOpType.add
                )
                nc.sync.dma_start(
                    out=y_out[ch * 128:(ch + 1) * 128, :], in_=y_sb[:]
                )

    nc.finalize()
    return nc


_PROG_CACHE = {}


def kernel(x, dijk, W1, b1, W2, b2, Win, Wout, bout, Wd, bd, idx_j, seg_i, seg_j):
    x = np.ascontiguousarray(np.asarray(x, dtype=np.float32))
    dijk = np.ascontiguousarray(np.asarray(dijk, dtype=np.float32))
    for b in (b1, b2, bout, bd):
        assert np.abs(np.asarray(b)).max() == 0.0, "nonzero biases unsupported"

    n_atoms, n_basis = x.shape
    n_edges, n_in = dijk.shape
    assert n_basis == 128 and np.asarray(W2).shape == (128, 128)

    p = Plan(n_atoms, n_edges, n_in)
    per_core = shard_inputs(p, x, dijk, idx_j, seg_i)

    key = (n_atoms, n_edges, n_in)
    if key not in _PROG_CACHE:
        _PROG_CACHE[key] = build_program(p)
    nc = _PROG_CACHE[key]

    common = dict(
        x_pad=_pad_rows(x, p.NX_PAD),
        w1b=np.asarray(W1, dtype=np.float32).astype(BF16),
        w2b=np.asarray(W2, dtype=np.float32).astype(BF16),
        winb=np.asarray(Win, dtype=np.float32).astype(BF16),
        woutb=np.asarray(Wout, dtype=np.float32).astype(BF16),
        wdb=np.asarray(Wd, dtype=np.float32).astype(BF16),
        ident_bf=np.eye(128, dtype=np.float32).astype(BF16),
        ident_f32=np.eye(128, dtype=np.float32),
    )
    in_maps = [{**common, **pc} for pc in per_core]
    res = run_bass_kernel_spmd(nc, in_maps, list(range(N_CORES)))
    global LAST_RESULTS
    LAST_RESULTS = res

    y = np.empty((n_atoms, 128), dtype=np.float32)
    v = np.empty((n_atoms, 128), dtype=np.float32)
    for c in range(N_CORES):
        y[c * p.NA:(c + 1) * p.NA] = res.results[c]["y_out"][: p.NA]
        v[c * p.NA:(c + 1) * p.NA] = res.results[c]["v_out"][: p.NA]
    return (y, v)



# revision 6
# speedup vs baseline: 6.3646x; 6.3646x over previous
"""CFNet interaction block on 8 trn2 NeuronCores (SPMD bass/tile kernel), v2.

Per core c of 8 (SPMD, one program, per-core data): core c owns atoms
[c*NA, (c+1)*NA) and the edges whose sorted seg_i lands there.

Host prep = pure layout (no reference FLOPs):
  - dijk cast fp32->bf16 and pre-TRANSPOSED into k-chunks [128|128|44, E_PC]
    (kills the device-side PE transposes and halves dijk HBM traffic),
  - x rows pre-gathered by idx_j, transposed: xg^T [128x, E_PC] bf16
    (kills the v1 per-edge dma_gather: ~9 ns of Q7 per edge),
  - one-hot S pages [T, 128, 128] bf16: edge row -> column (atom mod 128).

Static SPMD schedule: local atoms are split into 128-atom chunks; chunk k
gets a FIXED tile allotment TPW[k] (mean + 6 sigma), so every core's tile t
maps to the same chunk and the same psum window -- per-core variation is
absorbed by padding (~7% edge inflation).  Pad edges carry all-zero S rows.

Device pipeline per 512-edge block, [feature, edge] layout (weights are the
matmul stationaries):
  mm1  t1 = W1c.T @ dT (3 k-chunks)          psum [f1, e]
  ssp1 Exp (psum->sbuf, per block), Ln(0.5x+0.5) batched [128, 2048] -> bf16
  mm2  w^T = W2.T @ t1s                      psum [f2, e]
  sspw Exp per block, Ln batched             -> wt^T fp32
  mm_f f^T = Win.T @ xg^T                    psum [f, e]
  wf^T = wt^T * f^T (DVE)                    -> bf16
  PE-transpose wf^T -> wf [e, f] (psum bf16 -> sbuf)
  mm3 per 128-edge tile: conv^T[chunk] += wf_tile.T @ S_tile
       into a [128, 128] psum window per chunk; ~49 flushes to sbuf.
A single manual InstLoadActFuncSet(natural_log_exp_and_others) keeps Exp+Ln
resident: no ACT_TABLE_LOAD thrash (was 2.6 ms in v1).

Tail from sbuf-resident conv^T: z3^T = Wout.T @ conv^T, ssp, v^T = Wd.T @
h^T, y^T = v^T + x^T.  Outputs leave TRANSPOSED [128, NA_PAD]; the host
transposes back.  ssp(x) = Ln(0.5 + 0.5*Exp(x)) exactly.
"""

import math
import sys

import numpy as np
import ml_dtypes

sys.path.insert(0, "/opt/trn_rl_repo")

import concourse.bacc as bacc
import concourse.mybir as mybir
from concourse import tile
from concourse.bass_utils import run_bass_kernel_spmd

dt = mybir.dt
AF = mybir.ActivationFunctionType
BF16 = ml_dtypes.bfloat16

N_CORES = 8
TILE_E = 128            # edges per S tile / conv matmul
BLK = 512               # edges per pipeline block
GRP = 2048              # edges per DMA group (4 blocks, 16 tiles)
CHUNK_A = 128           # atoms per conv psum window
ACT_SET_LN_EXP = 6      # natural_log_exp_and_others in act_info.json


def _ceil(a, b):
    return -(-a // b)


def _to_bf16(a):
    """fp32 -> bf16 with round-to-nearest-even, fast numpy path."""
    a = np.ascontiguousarray(a, dtype=np.float32)
    v = a.view(np.uint32)
    r = ((v + np.uint32(0x7FFF) + ((v >> np.uint32(16)) & np.uint32(1)))
         >> np.uint32(16)).astype(np.uint16)
    return r.view(BF16).reshape(a.shape)


class Plan:
    """Shape-derived structure constants; identical for every core."""

    def __init__(self, n_atoms, n_edges, n_in):
        assert n_atoms % N_CORES == 0
        self.n_atoms, self.n_edges, self.n_in = n_atoms, n_edges, n_in
        self.NA = n_atoms // N_CORES
        epa = n_edges / n_atoms                      # mean edges per atom
        self.NCHUNK_REAL = _ceil(self.NA, CHUNK_A)
        tpw = []
        for k in range(self.NCHUNK_REAL):
            atoms_k = min(CHUNK_A, self.NA - CHUNK_A * k)
            mean = atoms_k * epa
            sig = math.sqrt(mean)
            tpw.append(int(_ceil(mean + 6.0 * sig, TILE_E)))
        t_raw = sum(tpw)
        self.T = int(_ceil(t_raw, GRP // TILE_E) * (GRP // TILE_E))
        self.TPW = tpw
        self.E_PC = self.T * TILE_E
        self.NBLK = self.E_PC // BLK
        self.NGRP = self.E_PC // GRP
        self.KC = [min(128, n_in - 128 * i) for i in range(_ceil(n_in, 128))]
        self.NKC = len(self.KC)
        # tile -> chunk map; trailing pad tiles attach to the last chunk
        self.first_tile = []
        self.tile_chunk = []
        for k, n in enumerate(tpw):
            self.first_tile.append(len(self.tile_chunk))
            self.tile_chunk += [k] * n
        self.tile_chunk += [self.NCHUNK_REAL - 1] * (self.T - t_raw)
        self.last_tile = [0] * self.NCHUNK_REAL
        for t, k in enumerate(self.tile_chunk):
            self.last_tile[k] = t
        self.NA_PAD = self.NCHUNK_REAL * CHUNK_A
        self.NSLAB = _ceil(self.NA_PAD, 512)         # tail slabs of 512 atoms
        self.NA_TAIL = self.NSLAB * 512


def shard_inputs(p, x, dijk_bf_T, xgT_all, seg_i):
    """Per-core layout prep. dijk_bf_T/xgT_all carry a zero pad column at
    index n_edges."""
    seg_i = np.asarray(seg_i).astype(np.int64)
    bounds = np.searchsorted(seg_i, np.arange(N_CORES + 1) * p.NA)
    ZCOL = p.n_edges                                  # the zero column

    per_core = []
    for c in range(N_CORES):
        lo, hi = int(bounds[c]), int(bounds[c + 1])
        es = seg_i[lo:hi] - c * p.NA                  # local atoms, sorted
        chunk = es // CHUNK_A

        # per-chunk edge placement into the static tile schedule
        cols = np.full(p.E_PC, ZCOL, dtype=np.int64)  # global edge id or pad
        s_t = np.empty(hi - lo, dtype=np.int64)       # tile of each edge
        s_r = np.empty(hi - lo, dtype=np.int64)       # row within tile
        cnt = np.bincount(chunk, minlength=p.NCHUNK_REAL)
        for k in range(p.NCHUNK_REAL):
            n_k = int(cnt[k])
            if n_k == 0:
                continue
            assert n_k <= p.TPW[k] * TILE_E, (c, k, n_k, p.TPW[k] * TILE_E)
            e0 = int(np.searchsorted(chunk, k))
            base = p.first_tile[k] * TILE_E
            pos = base + np.arange(n_k)
            cols[pos] = lo + e0 + np.arange(n_k)
            s_t[e0:e0 + n_k] = pos // TILE_E
            s_r[e0:e0 + n_k] = pos % TILE_E

        d = dijk_bf_T[:, cols]                        # [n_in, E_PC]
        d0 = np.ascontiguousarray(d[0:128])
        d1 = np.ascontiguousarray(d[128:256])
        d2 = np.ascontiguousarray(d[256:])
        xgT = np.ascontiguousarray(xgT_all[:, cols])  # [128, E_PC]

        S = np.zeros((p.T, TILE_E, CHUNK_A), dtype=BF16)
        s_c = es - chunk * CHUNK_A
        S[s_t, s_r, s_c] = 1.0

        xT = np.zeros((128, p.NA_TAIL), dtype=np.float32)
        xT[:, : p.NA] = x[c * p.NA : (c + 1) * p.NA].T

        per_core.append(dict(d0=d0, d1=d1, d2=d2, xgT=xgT, s_pages=S, xT=xT))
    return per_core


def build_program(p):
    nc = bacc.Bacc(None, target_bir_lowering=False)

    d0 = nc.declare_dram_parameter("d0", [128, p.E_PC], dt.bfloat16, isOutput=False)
    d1 = nc.declare_dram_parameter("d1", [128, p.E_PC], dt.bfloat16, isOutput=False)
    d2 = nc.declare_dram_parameter("d2", [p.KC[2], p.E_PC], dt.bfloat16, isOutput=False)
    xgT = nc.declare_dram_parameter("xgT", [128, p.E_PC], dt.bfloat16, isOutput=False)
    s_pages = nc.declare_dram_parameter(
        "s_pages", [p.T, TILE_E, CHUNK_A], dt.bfloat16, isOutput=False)
    xT = nc.declare_dram_parameter("xT", [128, p.NA_TAIL], dt.float32, isOutput=False)
    w1b = nc.declare_dram_parameter("w1b", [p.n_in, 128], dt.bfloat16, isOutput=False)
    w2b = nc.declare_dram_parameter("w2b", [128, 128], dt.bfloat16, isOutput=False)
    winb = nc.declare_dram_parameter("winb", [128, 128], dt.bfloat16, isOutput=False)
    woutb = nc.declare_dram_parameter("woutb", [128, 128], dt.bfloat16, isOutput=False)
    wdb = nc.declare_dram_parameter("wdb", [128, 128], dt.bfloat16, isOutput=False)
    identb = nc.declare_dram_parameter("identb", [128, 128], dt.bfloat16, isOutput=False)

    y_out = nc.declare_dram_parameter("y_out", [128, p.NA_TAIL], dt.float32, isOutput=True)
    v_out = nc.declare_dram_parameter("v_out", [128, p.NA_TAIL], dt.float32, isOutput=True)

    dsrc = [d0, d1, d2]

    with tile.TileContext(nc) as tc:
        # keep both Exp and Ln tables resident for the whole program
        nc.scalar.add_instruction(
            mybir.InstLoadActFuncSet(
                name=nc.get_next_instruction_name(), ins=[], outs=[],
                act_func_set_id=ACT_SET_LN_EXP,
            )
        )
        with (
            tc.tile_pool(name="const", bufs=1) as constp,
            tc.tile_pool(name="dload", bufs=2) as dload,
            tc.tile_pool(name="stage", bufs=2) as stage,
            tc.tile_pool(name="work", bufs=2) as work,
            tc.tile_pool(name="tail", bufs=2) as tailp,
            tc.tile_pool(name="ps_t1", bufs=2, space="PSUM") as ps_t1,
            tc.tile_pool(name="ps_w", bufs=2, space="PSUM") as ps_w,
            tc.tile_pool(name="ps_f", bufs=1, space="PSUM") as ps_f,
            tc.tile_pool(name="ps_tr", bufs=1, space="PSUM") as ps_tr,
            tc.tile_pool(name="ps_cv", bufs=2, space="PSUM") as ps_cv,
        ):
            # ---- constants ----
            idn = constp.tile([128, 128], dt.bfloat16)
            nc.sync.dma_start(out=idn[:], in_=identb[:, :])
            half_c = constp.tile([128, 1], dt.float32)
            nc.gpsimd.memset(half_c[:], 0.5)
            w1sb = []
            for kc in range(p.NKC):
                kn = p.KC[kc]
                t = constp.tile([128, 128], dt.bfloat16, name=f"w1sb{kc}")
                nc.sync.dma_start(out=t[:kn, :], in_=w1b[kc * 128: kc * 128 + kn, :])
                w1sb.append(t)
            w2sb = constp.tile([128, 128], dt.bfloat16)
            nc.sync.dma_start(out=w2sb[:], in_=w2b[:, :])
            winsb = constp.tile([128, 128], dt.bfloat16)
            nc.sync.dma_start(out=winsb[:], in_=winb[:, :])
            woutsb = constp.tile([128, 128], dt.bfloat16)
            nc.sync.dma_start(out=woutsb[:], in_=woutb[:, :])
            wdsb = constp.tile([128, 128], dt.bfloat16)
            nc.sync.dma_start(out=wdsb[:], in_=wdb[:, :])
            xT_sb = constp.tile([128, p.NA_TAIL], dt.float32)
            nc.sync.dma_start(out=xT_sb[:], in_=xT[:, :])
            convT = constp.tile([128, p.NA_TAIL], dt.bfloat16)

            conv_tiles = {}

            # ---- edge pipeline ----
            for g in range(p.NGRP):
                e0 = g * GRP
                dg = []
                for kc in range(p.NKC):
                    kn = p.KC[kc]
                    tdg = dload.tile([kn, GRP], dt.bfloat16, tag=f"dg{kc}",
                                     name=f"dg{kc}")
                    nc.sync.dma_start(out=tdg[:], in_=dsrc[kc][:, e0:e0 + GRP])
                    dg.append(tdg)
                xgg = dload.tile([128, GRP], dt.bfloat16, tag="xgg")
                nc.sync.dma_start(out=xgg[:], in_=xgT[:, e0:e0 + GRP])
                sg = dload.tile([128, 16, CHUNK_A], dt.bfloat16, tag="sg")
                nc.sync.dma_start(
                    out=sg[:],
                    in_=s_pages[g * 16:(g + 1) * 16, :, :].rearrange(
                        "t pp c -> pp t c", pp=128),
                )

                # phase 1: mm1 + Exp per block; Ln batched over the group
                e1g = stage.tile([128, 4, BLK], dt.float32, tag="e1g")
                for b in range(4):
                    t1 = ps_t1.tile([128, BLK], dt.float32, tag="t1")
                    for kc in range(p.NKC):
                        kn = p.KC[kc]
                        nc.tensor.matmul(
                            t1[:], w1sb[kc][:kn, :],
                            dg[kc][:, b * BLK:(b + 1) * BLK],
                            start=(kc == 0), stop=(kc == p.NKC - 1),
                        )
                    nc.scalar.activation(e1g[:, b, :], t1[:], AF.Exp)
                t1sg = stage.tile([128, 4, BLK], dt.bfloat16, tag="t1sg")
                nc.scalar.activation(
                    t1sg[:], e1g[:], AF.Ln, bias=half_c[:], scale=half_c[:])

                # phase 2: mm2 + Exp per block; Ln batched
                ewg = stage.tile([128, 4, BLK], dt.float32, tag="ewg")
                for b in range(4):
                    wps = ps_w.tile([128, BLK], dt.float32, tag="wps")
                    nc.tensor.matmul(
                        wps[:], w2sb[:], t1sg[:, b, :], start=True, stop=True)
                    nc.scalar.activation(ewg[:, b, :], wps[:], AF.Exp)
                wtg = stage.tile([128, 4, BLK], dt.float32, tag="wtg")
                nc.scalar.activation(
                    wtg[:], ewg[:], AF.Ln, bias=half_c[:], scale=half_c[:])

                # phase 3: mm_f, wf, transpose, mm3 per block
                for b in range(4):
                    fps = ps_f.tile([128, BLK], dt.float32, tag="fps")
                    nc.tensor.matmul(
                        fps[:], winsb[:], xgg[:, b * BLK:(b + 1) * BLK],
                        start=True, stop=True)
                    wfT = work.tile([128, BLK], dt.bfloat16, tag="wfT")
                    nc.vector.tensor_tensor(
                        wfT[:], wtg[:, b, :], fps[:], mybir.AluOpType.mult)
                    wfP = ps_tr.tile([128, BLK], dt.bfloat16, tag="wfP")
                    for i in range(4):
                        nc.tensor.transpose(
                            wfP[:, i * 128:(i + 1) * 128],
                            wfT[:, i * 128:(i + 1) * 128], idn[:])
                    wf = work.tile([128, BLK], dt.bfloat16, tag="wf")
                    nc.vector.tensor_copy(wf[:], wfP[:])

                    for i in range(4):
                        t = g * 16 + b * 4 + i           # global tile id
                        k = p.tile_chunk[t]
                        if p.first_tile[k] == t:
                            cv = ps_cv.tile([128, CHUNK_A], dt.float32,
                                            tag="cv", name="cv")
                            nc.vector.memset(cv[:], 0.0)
                            conv_tiles[k] = cv
                        cv = conv_tiles[k]
                        nc.tensor.matmul(
                            cv[:], wf[:, i * 128:(i + 1) * 128],
                            sg[:, b * 4 + i, :],
                            start=False, stop=(p.last_tile[k] == t),
                            skip_group_check=True,
                        )
                        if p.last_tile[k] == t:
                            nc.vector.tensor_copy(
                                convT[:, k * CHUNK_A:(k + 1) * CHUNK_A], cv[:])
                            del conv_tiles[k]

            # ---- tail: z3^T = Wout.T @ conv^T, ssp, v^T, y^T ----
            for s in range(p.NSLAB):
                a0 = s * 512
                z3 = ps_t1.tile([128, 512], dt.float32, tag="t1", name="z3")
                nc.tensor.matmul(
                    z3[:], woutsb[:], convT[:, a0:a0 + 512], start=True, stop=True)
                e3 = tailp.tile([128, 512], dt.float32, tag="e3")
                nc.scalar.activation(e3[:], z3[:], AF.Exp)
                hT = tailp.tile([128, 512], dt.bfloat16, tag="hT")
                nc.scalar.activation(
                    hT[:], e3[:], AF.Ln, bias=half_c[:], scale=half_c[:])
                vps = ps_w.tile([128, 512], dt.float32, tag="wps", name="vps")
                nc.tensor.matmul(vps[:], wdsb[:], hT[:], start=True, stop=True)
                v_sb = tailp.tile([128, 512], dt.float32, tag="v_sb")
                nc.vector.tensor_copy(v_sb[:], vps[:])
                nc.sync.dma_start(out=v_out[:, a0:a0 + 512], in_=v_sb[:])
                y_sb = tailp.tile([128, 512], dt.float32, tag="y_sb")
                nc.vector.tensor_tensor(
                    y_sb[:], vps[:], xT_sb[:, a0:a0 + 512], mybir.AluOpType.add)
                nc.sync.dma_start(out=y_out[:, a0:a0 + 512], in_=y_sb[:])

    nc.finalize()
    return nc


_PROG_CACHE = {}


def kernel(x, dijk, W1, b1, W2, b2, Win, Wout, bout, Wd, bd, idx_j, seg_i, seg_j):
    x = np.ascontiguousarray(np.asarray(x, dtype=np.float32))
    dijk = np.ascontiguousarray(np.asarray(dijk, dtype=np.float32))
    for b in (b1, b2, bout, bd):
        assert np.abs(np.asarray(b)).max() == 0.0, "nonzero biases unsupported"

    n_atoms, n_basis = x.shape
    n_edges, n_in = dijk.shape
    assert n_basis == 128 and np.asarray(W2).shape == (128, 128)

    p = Plan(n_atoms, n_edges, n_in)

    # global host-side layout transforms (shared across cores)
    dijk_bf_T = np.zeros((n_in, n_edges + 1), dtype=BF16)
    dijk_bf_T[:, :n_edges] = _to_bf16(dijk).T
    x_bf = _to_bf16(x)
    idx = np.asarray(idx_j).astype(np.int64)
    xgT_all = np.zeros((128, n_edges + 1), dtype=BF16)
    xgT_all[:, :n_edges] = x_bf[idx].T

    per_core = shard_inputs(p, x, dijk_bf_T, xgT_all, seg_i)
    del dijk_bf_T, xgT_all

    key = (n_atoms, n_edges, n_in)
    if key not in _PROG_CACHE:
        _PROG_CACHE[key] = build_program(p)
    nc = _PROG_CACHE[key]

    common = dict(
        w1b=_to_bf16(np.asarray(W1, dtype=np.float32)),
        w2b=_to_bf16(np.asarray(W2, dtype=np.float32)),
        winb=_to_bf16(np.asarray(Win, dtype=np.float32)),
        woutb=_to_bf16(np.asarray(Wout, dtype=np.float32)),
        wdb=_to_bf16(np.asarray(Wd, dtype=np.float32)),
        identb=_to_bf16(np.eye(128, dtype=np.float32)),
    )
    in_maps = [{**common, **pc} for pc in per_core]
    res = run_bass_kernel_spmd(nc, in_maps, list(range(N_CORES)))
    global LAST_RESULTS
    LAST_RESULTS = res

    y = np.empty((n_atoms, 128), dtype=np.float32)
    v = np.empty((n_atoms, 128), dtype=np.float32)
    for c in range(N_CORES):
        y[c * p.NA:(c + 1) * p.NA] = res.results[c]["y_out"][:, : p.NA].T
        v[c * p.NA:(c + 1) * p.NA] = res.results[c]["v_out"][:, : p.NA].T
    return (y, v)
